# revision 24
# baseline (speedup 1.0000x reference)
"""Tensor-parallel LlamaAttention (S=2048, HID=4096, NH=32, NKV=8) on 8 trn2 cores.

Sharding: core c owns q heads {c, c+8, c+16, c+24} (all share kv head c) and
kv head c.  Projections + attention fully local; per q-chunk C the normalized
avT slices of all 4 local heads are AllGathered (two half-gathers, j01/j23),
then each core computes its 512 o_proj output columns for that chunk's 4 seq
tiles (column-parallel wo) — so o_proj overlaps later attention chunks and
only the last chunk's gather is exposed.

Host packs all inputs partition-major so DMA lines are 8-32KB.
"""

import numpy as np
import ml_dtypes

import concourse.bacc as bacc
import concourse.tile as tile
import concourse.mybir as mybir
from concourse.bass_utils import run_bass_kernel_spmd

S = 2048
HID = 4096
NH = 32
NKV = 8
HD = 128
HALF = 64
N_CORES = 8
NREP = NH // NKV  # 4 q heads per core
NHT = HID // 128  # 32 hidden tiles
NST = S // 128    # 16 seq tiles
NSC = S // 512    # 4 seq chunks
BF16 = mybir.dt.bfloat16
F32 = mybir.dt.float32
F32R = mybir.dt.float32r

_CACHE = {}


def build_nc():
    nc = bacc.Bacc("TRN2", target_bir_lowering=False, debug=False,
                   num_devices=N_CORES)

    xP = nc.dram_tensor("xP", [128, NSC * NHT * 512], BF16,
                        kind="ExternalInput").ap()
    wqP = nc.dram_tensor("wqP", [128, NHT * 512], BF16,
                         kind="ExternalInput").ap()
    wkP = nc.dram_tensor("wkP", [128, NHT * 128], BF16,
                         kind="ExternalInput").ap()
    wvP = nc.dram_tensor("wvP", [128, NHT * 128], BF16,
                         kind="ExternalInput").ap()
    woP = nc.dram_tensor("woP", [128, NHT * 512], BF16,
                         kind="ExternalInput").ap()
    cosP = nc.dram_tensor("cosP", [HD, S], BF16, kind="ExternalInput").ap()
    sinmP = nc.dram_tensor("sinmP", [HD, S], BF16, kind="ExternalInput").ap()
    tri = nc.dram_tensor("triT", [128, 128], BF16, kind="ExternalInput").ap()
    eye = nc.dram_tensor("eyeT", [128, 128], BF16, kind="ExternalInput").ap()
    ones_c = nc.dram_tensor("ones_c", [128, 1], BF16, kind="ExternalInput").ap()
    ones_r = nc.dram_tensor("ones_r", [1, 128], F32, kind="ExternalInput").ap()

    o_out = nc.dram_tensor("o_out", [S, 512], F32, kind="ExternalOutput").ap()

    ag_in = [nc.dram_tensor(f"ag_in{c}", [128, 2048], BF16).ap()
             for c in range(NSC)]
    ag_out = [nc.dram_tensor(f"ag_out{c}", [N_CORES * 128, 2048], BF16,
                             addr_space="Shared").ap() for c in range(NSC)]

    with tile.TileContext(nc) as tc:
        _body(nc, tc, xP, wqP, wkP, wvP, woP, cosP, sinmP, tri, eye,
              ones_c, ones_r, o_out, ag_in, ag_out)
    nc.compile()
    return nc


def _body(nc, tc, xP, wqP, wkP, wvP, woP, cosP, sinmP, tri, eye,
          ones_c, ones_r, o_out, ag_in, ag_out):
    with tc.tile_pool(name="consts", bufs=1) as cpool:
        tri_sb = cpool.tile([128, 128], BF16, tag="tri")
        eye_sb = cpool.tile([128, 128], BF16, tag="eye")
        onc_sb = cpool.tile([128, 1], BF16, tag="onc")
        onr_sb = cpool.tile([1, 128], F32, tag="onr")
        nc.sync.dma_start(out=tri_sb[:], in_=tri[:])
        nc.sync.dma_start(out=eye_sb[:], in_=eye[:])
        nc.sync.dma_start(out=onc_sb[:], in_=ones_c[:])
        nc.sync.dma_start(out=onr_sb[:], in_=ones_r[:])

        with tc.tile_pool(name="qkv", bufs=1) as qkvpool:
            qT_sb = [qkvpool.tile([HD, S], BF16, tag=f"qT{j}", name=f"qT{j}")
                     for j in range(NREP)]
            kT_sb = qkvpool.tile([HD, S], BF16, tag="kT")
            v_sb = qkvpool.tile([128, S], BF16, tag="v")  # [s-in-tile, d]

            with (
                tc.tile_pool(name="rconsts", bufs=1) as rcpool,
                tc.tile_pool(name="wproj", bufs=1) as wpool,
                tc.tile_pool(name="xc", bufs=6) as xpool,
                tc.tile_pool(name="rope", bufs=2) as rpool,
                tc.tile_pool(name="pmm1", bufs=6, space="PSUM") as pmm1,
                tc.tile_pool(name="ptr", bufs=2, space="PSUM") as ptrp,
            ):
                _phase1(nc, tc, xP, wqP, wkP, wvP, cosP, sinmP, eye_sb,
                        qT_sb, kT_sb, v_sb, rcpool, wpool, xpool, rpool,
                        pmm1, ptrp)

            with (
                tc.tile_pool(name="wo", bufs=1) as wopool,
                tc.tile_pool(name="av", bufs=2) as avpool,
                tc.tile_pool(name="probs", bufs=16) as ppool,
                tc.tile_pool(name="srec", bufs=2) as srpool,
                tc.tile_pool(name="sbrd", bufs=2) as sbpool,
                tc.tile_pool(name="ag", bufs=16) as agpool,
                tc.tile_pool(name="oout", bufs=4) as opool,
                tc.tile_pool(name="pmm2", bufs=4, space="PSUM") as pmm2,
                tc.tile_pool(name="psav", bufs=2, space="PSUM") as psav,
                tc.tile_pool(name="psrs", bufs=2, space="PSUM") as psrs,
            ):
                _phase23(nc, tc, woP, o_out, ag_in, ag_out,
                         tri_sb, onc_sb, onr_sb,
                         qT_sb, kT_sb, v_sb, wopool, avpool, ppool,
                         srpool, sbpool, agpool, opool, pmm2, psav, psrs)


def _phase1(nc, tc, xP, wqP, wkP, wvP, cosP, sinmP, eye_sb,
            qT_sb, kT_sb, v_sb, rcpool, wpool, xpool, rpool, pmm1, ptrp):
    # one dma_start per tensor: its descriptors fan out across all 16 DMA
    # rings, and sync-engine issue cost stays minimal (the scalar engine's
    # queue stalls DMAs behind compute-dependent copies).
    cos_sb = rcpool.tile([HD, S], BF16, tag="cos")
    sinm_sb = rcpool.tile([HD, S], BF16, tag="sinm")
    nc.sync.dma_start(out=cos_sb[:], in_=cosP[:])
    nc.sync.dma_start(out=sinm_sb[:], in_=sinmP[:])

    wq_sb = wpool.tile([128, NHT * 512], BF16, tag="wq")
    wk_sb = wpool.tile([128, NHT * 128], BF16, tag="wk")
    wv_sb = wpool.tile([128, NHT * 128], BF16, tag="wv")
    nc.sync.dma_start(out=wq_sb[:], in_=wqP[:])
    nc.sync.dma_start(out=wk_sb[:], in_=wkP[:])
    nc.sync.dma_start(out=wv_sb[:], in_=wvP[:])

    def _rope(dst, pp, sc):
        # dst[:, sc] = rot(pp): [0:64] = x1*c - x2*s ; [64:128] = x2*c + x1*s
        # cos_sb = [c; c], sinm_sb = [-s; s] (signs baked host-side)
        pq_sb = rpool.tile([128, 512], BF16, tag="pq")
        nc.scalar.copy(pq_sb[:], pp[:])
        sw = rpool.tile([128, 512], BF16, tag="sw")
        nc.vector.tensor_scalar_mul(sw[0:HALF, :], pq_sb[HALF:128, :], 1.0)
        nc.vector.tensor_scalar_mul(sw[HALF:128, :], pq_sb[0:HALF, :], 1.0)
        a = rpool.tile([128, 512], BF16, tag="ra")
        b = rpool.tile([128, 512], BF16, tag="rb")
        nc.vector.tensor_mul(a[:], pq_sb[:], cos_sb[:, sc])
        nc.vector.tensor_mul(b[:], sw[:], sinm_sb[:, sc])
        nc.vector.tensor_add(dst[:, sc], a[:], b[:])

    HW = NHT * 256  # 8192 cols per half-chunk
    for cs in range(NSC):
        sc = slice(cs * 512, (cs + 1) * 512)
        xh = [xpool.tile([128, HW], BF16, tag="x", name=f"x{cs}_{hf}")
              for hf in range(2)]
        for hf in range(2):
            nc.sync.dma_start(out=xh[hf][:],
                              in_=xP[:, (2 * cs + hf) * HW:
                                     (2 * cs + hf + 1) * HW])

        def xs(h):
            return xh[h // 16][:, (h % 16) * 512:(h % 16 + 1) * 512]

        for j in range(NREP):
            pq = pmm1.tile([128, 512], F32, tag="mm", name=f"pq{cs}_{j}")
            for h in range(NHT):
                nc.tensor.matmul(
                    pq[:],
                    wq_sb[:, h * 512 + j * 128: h * 512 + (j + 1) * 128],
                    xs(h),
                    start=(h == 0), stop=(h == NHT - 1))
            _rope(qT_sb[j], pq, sc)

        pk = pmm1.tile([128, 512], F32, tag="mm", name=f"pk{cs}")
        for h in range(NHT):
            nc.tensor.matmul(pk[:], wk_sb[:, h * 128:(h + 1) * 128],
                             xs(h),
                             start=(h == 0), stop=(h == NHT - 1))
        _rope(kT_sb, pk, sc)

        # vT chunk [d, s'] with N=512 matmuls, then PE-transpose to [s, d]
        pv = pmm1.tile([128, 512], F32, tag="mm", name=f"pv{cs}")
        for h in range(NHT):
            nc.tensor.matmul(pv[:], wv_sb[:, h * 128:(h + 1) * 128],
                             xs(h),
                             start=(h == 0), stop=(h == NHT - 1))
        vt_sb = rpool.tile([128, 512], BF16, tag="vt", name=f"vt{cs}")
        nc.scalar.copy(vt_sb[:], pv[:])
        for tl in range(4):
            pt_ps = ptrp.tile([128, 128], BF16, tag="tr", name=f"vtr{cs}_{tl}")
            nc.tensor.transpose(pt_ps[:], vt_sb[:, tl * 128:(tl + 1) * 128],
                                eye_sb[:])
            t0 = (cs * 4 + tl) * 128
            nc.scalar.copy(v_sb[:, t0:t0 + 128], pt_ps[:])


def _phase23(nc, tc, woP, o_out, ag_in, ag_out,
             tri_sb, onc_sb, onr_sb, qT_sb, kT_sb, v_sb, wopool, avpool,
             ppool, srpool, sbpool, agpool, opool, pmm2, psav, psrs):
    Exp = mybir.ActivationFunctionType.Exp
    DEPTH = 3
    RG = [list(range(N_CORES))]

    wo_sb = wopool.tile([128, NHT * 512], BF16, tag="wo")
    nc.sync.dma_start(out=wo_sb[:], in_=woP[:])

    av_tiles = [avpool.tile([128, 2048], BF16, tag="av", name=f"av{C}")
                for C in range(NSC)]

    state = {"recip": None, "norm": None}
    rrecs = {}

    def make_recip(j, C, prs):
        def f():
            rrec = srpool.tile([1, 512], F32, tag="rrec", name=f"rrec{j}_{C}")
            nc.vector.reciprocal_approx_fast(rrec[:], prs[:])
            rrecs[(j, C)] = rrec
        return f

    def make_norm(j, C, pav):
        def f():
            rrec = rrecs.pop((j, C))
            pb = pmm2.tile([128, 512], F32, tag="mm", name=f"pb{j}_{C}")
            nc.tensor.matmul(pb[:], onr_sb[:], rrec[:], start=True, stop=True)
            bsb = sbpool.tile([128, 512], F32, tag="bsb", name=f"bsb{j}_{C}")
            nc.vector.tensor_scalar_mul(bsb[:], pb[:], 1.0)
            nc.vector.tensor_mul(av_tiles[C][:, j * 512:(j + 1) * 512],
                                 pav[:], bsb[:])
            if j == 3:
                # one AllGather per chunk: collectives serialize on the
                # gpsimd queue, so fewer+bigger is faster
                nc.sync.dma_start(out=ag_in[C][:], in_=av_tiles[C][:])
                nc.gpsimd.collective_compute(
                    "AllGather", mybir.AluOpType.bypass, replica_groups=RG,
                    ins=[ag_in[C][:]], outs=[ag_out[C][:]])
        return f

    def run_carry(kind):
        if state[kind] is not None:
            state[kind]()
            state[kind] = None

    def attn_unit(j, C):
        qc0 = C * 512
        nkt = 4 * C + 4
        pav = psav.tile([128, 512], F32, tag="av", name=f"pav{j}_{C}")
        prs = psrs.tile([1, 512], F32, tag="rs", name=f"prs{j}_{C}")
        pend = []

        def drain_one():
            kt2, pt2 = pend.pop(0)
            nc.tensor.matmul(prs[:], onc_sb[:], pt2[:],
                             start=(kt2 == 0), stop=(kt2 == nkt - 1))
            nc.tensor.matmul(pav[:], v_sb[:, kt2 * 128:(kt2 + 1) * 128],
                             pt2[:], start=(kt2 == 0), stop=(kt2 == nkt - 1))

        for kt in range(nkt):
            off = max(0, (kt - 4 * C) * 128)
            ps = pmm2.tile([128, 512], F32, tag="mm", name=f"ps{j}_{C}_{kt}")
            nc.tensor.matmul(ps[:, off:512],
                             kT_sb[:, kt * 128:(kt + 1) * 128],
                             qT_sb[j][:, qc0 + off: qc0 + 512],
                             start=True, stop=True)
            pt = ppool.tile([128, 512], BF16, tag="pt", name=f"pt{j}_{C}_{kt}")
            if off > 0:
                nc.vector.memset(pt[:, 0:off], 0.0)
            nc.scalar.activation(pt[:, off:512], ps[:, off:512], Exp)
            if kt >= 4 * C:
                nc.vector.tensor_mul(pt[:, off:off + 128],
                                     pt[:, off:off + 128], tri_sb[:])
            pend.append((kt, pt))
            if kt == 1:
                run_carry("recip")
            if kt == 3:
                run_carry("norm")
            if len(pend) > DEPTH:
                drain_one()
        while pend:
            drain_one()
        state["recip"] = make_recip(j, C, prs)
        state["norm"] = make_norm(j, C, pav)

    def ph3(C):
        # readback 8 slabs [128, 2048]; slab r holds (j, r) tiles at
        # cols j*512 + q'; one sync dma_start each (16KB lines over rings)
        ts = []
        for r in range(N_CORES):
            t = agpool.tile([128, 2048], BF16, tag="ag", name=f"ag{C}_{r}")
            nc.sync.dma_start(out=t[:],
                              in_=ag_out[C][r * 128:(r + 1) * 128, :])
            ts.append(t)
        for stl in range(4):
            st = 4 * C + stl
            po = pmm2.tile([128, 512], F32, tag="mm", name=f"po{st}")
            n = 0
            for jgrp in range(NREP):
                for r in range(N_CORES):
                    nc.tensor.matmul(
                        po[:],
                        ts[r][:, jgrp * 512 + stl * 128:
                              jgrp * 512 + (stl + 1) * 128],
                        wo_sb[:, (jgrp * 8 + r) * 512:(jgrp * 8 + r + 1) * 512],
                        start=(n == 0), stop=(n == NREP * N_CORES - 1))
                    n += 1
            osb = opool.tile([128, 512], F32, tag="o", name=f"o{st}")
            nc.vector.tensor_scalar_mul(osb[:], po[:], 1.0)
            nc.sync.dma_start(out=o_out[st * 128:(st + 1) * 128, :],
                              in_=osb[:])

    # AG(C) fires in unit (C+1, 0)'s norm carry; the 4 AGs serialize on
    # gpsimd (~28us each), completing at roughly 44/72/100/128us into
    # phase 2 — consume each only after it has landed.
    PH3_AT = {(2, 2): 0, (3, 1): 1}
    for C in range(NSC):
        for j in range(NREP):
            attn_unit(j, C)
            if (C, j) in PH3_AT:
                ph3(PH3_AT[(C, j)])
    run_carry("recip")
    run_carry("norm")
    ph3(NSC - 2)
    ph3(NSC - 1)


def prep_inputs(hidden_states, wq, wk, wv, wo, cos, sin, causal_mask=None):
    bf16 = ml_dtypes.bfloat16
    x = np.asarray(hidden_states, np.float32)[0]          # (S, HID)
    xT = np.ascontiguousarray(x.T)                        # (HID, S)
    # chunk-major pack: xP[p, cs*16384 + h*512 + s'] = xT[h*128+p, cs*512+s']
    xP = np.ascontiguousarray(
        xT.reshape(NHT, 128, NSC, 512).transpose(1, 2, 0, 3)
        .reshape(128, NSC * NHT * 512)).astype(bf16)
    wq_s = np.asarray(wq, np.float32) / np.sqrt(HD)
    cos2 = np.asarray(cos, np.float32)[0, 0]              # (S, 64)
    sin2 = np.asarray(sin, np.float32)[0, 0]
    cosP = np.ascontiguousarray(
        np.concatenate([cos2.T, cos2.T], 0)).astype(bf16)  # [c; c]
    sinmP = np.ascontiguousarray(
        np.concatenate([-sin2.T, sin2.T], 0)).astype(bf16)  # [-s; s]
    kl = np.arange(128)[:, None]
    ql = np.arange(128)[None, :]
    triT = (kl <= ql).astype(bf16)
    eyeT = np.eye(128, dtype=np.float32).astype(bf16)
    ones_c = np.ones((128, 1), bf16)
    ones_r = np.ones((1, 128), np.float32)

    # wo reordered so row p = (j*8+r)*128 + d maps head (j*8+r), dim d
    j_ = np.arange(NREP)[:, None, None]
    r_ = np.arange(N_CORES)[None, :, None]
    d_ = np.arange(HD)[None, None, :]
    col_order = ((j_ * N_CORES + r_) * HD + d_).reshape(-1)
    woT_full = np.ascontiguousarray(
        np.asarray(wo, np.float32)[:, col_order].T)       # (4096 avrow, 4096)

    def pack_w(wT, ncols):  # (HID, ncols) -> (128, NHT*ncols), col h*ncols+c
        return np.ascontiguousarray(
            wT.reshape(NHT, 128, ncols).transpose(1, 0, 2)
            .reshape(128, NHT * ncols)).astype(bf16)

    in_maps = []
    for c in range(N_CORES):
        heads = [jj * N_CORES + c for jj in range(NREP)]
        wq_rows = np.concatenate([wq_s[h * HD:(h + 1) * HD, :] for h in heads],
                                 0)                        # (512, HID)
        wqP = pack_w(np.ascontiguousarray(wq_rows.T), 512)
        wkT_c = np.ascontiguousarray(
            np.asarray(wk, np.float32)[c * HD:(c + 1) * HD, :].T)
        wvT_c = np.ascontiguousarray(
            np.asarray(wv, np.float32)[c * HD:(c + 1) * HD, :].T)
        wkP = pack_w(wkT_c, 128)
        wvP = pack_w(wvT_c, 128)
        woP = pack_w(np.ascontiguousarray(
            woT_full[:, c * 512:(c + 1) * 512]), 512)
        in_maps.append(dict(xP=xP, wqP=wqP, wkP=wkP, wvP=wvP, woP=woP,
                            cosP=cosP, sinmP=sinmP, triT=triT, eyeT=eyeT,
                            ones_c=ones_c, ones_r=ones_r))
    return in_maps


def postprocess(results):
    out = np.empty((S, HID), np.float32)
    for c in range(N_CORES):
        out[:, c * 512:(c + 1) * 512] = results[c]["o_out"]
    return out[None]


def get_nc():
    if "nc" not in _CACHE:
        _CACHE["nc"] = build_nc()
    return _CACHE["nc"]


def kernel(hidden_states, wq, wk, wv, wo, cos, sin, causal_mask=None):
    nc = get_nc()
    in_maps = prep_inputs(hidden_states, wq, wk, wv, wo, cos, sin, causal_mask)
    res = run_bass_kernel_spmd(nc, in_maps, core_ids=list(range(N_CORES)))
    return postprocess(res.results)


# revision 32
# speedup vs baseline: 1.0114x; 1.0114x over previous
"""Tensor-parallel LlamaAttention (S=2048, HID=4096, NH=32, NKV=8) on 8 trn2 cores.

Sharding: core c owns q heads {c, c+8, c+16, c+24} (all share kv head c) and
kv head c.  Projections + attention fully local; per q-chunk C the normalized
avT slices of all 4 local heads are AllGathered (two half-gathers, j01/j23),
then each core computes its 512 o_proj output columns for that chunk's 4 seq
tiles (column-parallel wo) — so o_proj overlaps later attention chunks and
only the last chunk's gather is exposed.

Host packs all inputs partition-major so DMA lines are 8-32KB.
"""

import numpy as np
import ml_dtypes

import concourse.bacc as bacc
import concourse.tile as tile
import concourse.mybir as mybir
from concourse.bass_utils import run_bass_kernel_spmd

S = 2048
HID = 4096
NH = 32
NKV = 8
HD = 128
HALF = 64
N_CORES = 8
NREP = NH // NKV  # 4 q heads per core
NHT = HID // 128  # 32 hidden tiles
NST = S // 128    # 16 seq tiles
NSC = S // 512    # 4 seq chunks
BF16 = mybir.dt.bfloat16
F32 = mybir.dt.float32
F32R = mybir.dt.float32r

_CACHE = {}


def build_nc():
    nc = bacc.Bacc("TRN2", target_bir_lowering=False, debug=False,
                   num_devices=N_CORES)

    xP = nc.dram_tensor("xP", [128, NSC * NHT * 512], BF16,
                        kind="ExternalInput").ap()
    wqP = nc.dram_tensor("wqP", [128, NHT * 512], BF16,
                         kind="ExternalInput").ap()
    wkP = nc.dram_tensor("wkP", [128, NHT * 128], BF16,
                         kind="ExternalInput").ap()
    wvP = nc.dram_tensor("wvP", [128, NHT * 128], BF16,
                         kind="ExternalInput").ap()
    woP = nc.dram_tensor("woP", [128, NHT * 512], BF16,
                         kind="ExternalInput").ap()
    cosP = nc.dram_tensor("cosP", [HD, S], BF16, kind="ExternalInput").ap()
    sinmP = nc.dram_tensor("sinmP", [HD, S], BF16, kind="ExternalInput").ap()
    tri = nc.dram_tensor("triT", [128, 128], BF16, kind="ExternalInput").ap()
    eye = nc.dram_tensor("eyeT", [128, 128], BF16, kind="ExternalInput").ap()
    ones_c = nc.dram_tensor("ones_c", [128, 1], BF16, kind="ExternalInput").ap()
    ones_r = nc.dram_tensor("ones_r", [1, 128], F32, kind="ExternalInput").ap()

    o_out = nc.dram_tensor("o_out", [S, 512], F32, kind="ExternalOutput").ap()

    ag_in = [nc.dram_tensor(f"ag_in{c}", [128, 2048], BF16).ap()
             for c in range(NSC)]
    ag_out = [nc.dram_tensor(f"ag_out{c}", [N_CORES * 128, 2048], BF16,
                             addr_space="Shared").ap() for c in range(NSC)]

    with tile.TileContext(nc) as tc:
        _body(nc, tc, xP, wqP, wkP, wvP, woP, cosP, sinmP, tri, eye,
              ones_c, ones_r, o_out, ag_in, ag_out)
    nc.compile()
    return nc


def _body(nc, tc, xP, wqP, wkP, wvP, woP, cosP, sinmP, tri, eye,
          ones_c, ones_r, o_out, ag_in, ag_out):
    with tc.tile_pool(name="consts", bufs=1) as cpool:
        tri_sb = cpool.tile([128, 128], BF16, tag="tri")
        eye_sb = cpool.tile([128, 128], BF16, tag="eye")
        onc_sb = cpool.tile([128, 1], F32, tag="onc")
        onr_sb = cpool.tile([1, 128], BF16, tag="onr")
        nc.sync.dma_start(out=tri_sb[:], in_=tri[:])
        nc.sync.dma_start(out=eye_sb[:], in_=eye[:])
        nc.gpsimd.memset(onc_sb[:], 1.0)
        nc.gpsimd.memset(onr_sb[:], 1.0)

        with tc.tile_pool(name="qkv", bufs=1) as qkvpool:
            qT_sb = [qkvpool.tile([HD, S], BF16, tag=f"qT{j}", name=f"qT{j}")
                     for j in range(NREP)]
            kT_sb = qkvpool.tile([HD, S], BF16, tag="kT")
            v_sb = qkvpool.tile([128, S], BF16, tag="v")  # [s-in-tile, d]

            with (
                tc.tile_pool(name="rconsts", bufs=1) as rcpool,
                tc.tile_pool(name="wproj", bufs=1) as wpool,
                tc.tile_pool(name="xc", bufs=6) as xpool,
                tc.tile_pool(name="rope", bufs=2) as rpool,
                tc.tile_pool(name="pmm1", bufs=6, space="PSUM") as pmm1,
                tc.tile_pool(name="ptr", bufs=2, space="PSUM") as ptrp,
            ):
                _phase1(nc, tc, xP, wqP, wkP, wvP, cosP, sinmP, eye_sb,
                        qT_sb, kT_sb, v_sb, rcpool, wpool, xpool, rpool,
                        pmm1, ptrp)

            with (
                tc.tile_pool(name="wo", bufs=1) as wopool,
                tc.tile_pool(name="av", bufs=2) as avpool,
                tc.tile_pool(name="probs", bufs=16) as ppool,
                tc.tile_pool(name="srec", bufs=2) as srpool,
                tc.tile_pool(name="sbrd", bufs=2) as sbpool,
                tc.tile_pool(name="tsum", bufs=2) as tspool,
                tc.tile_pool(name="ag", bufs=16) as agpool,
                tc.tile_pool(name="oout", bufs=4) as opool,
                tc.tile_pool(name="pmm2", bufs=4, space="PSUM") as pmm2,
                tc.tile_pool(name="psav", bufs=2, space="PSUM") as psav,
                tc.tile_pool(name="psrs", bufs=2, space="PSUM") as psrs,
            ):
                _phase23(nc, tc, woP, o_out, ag_in, ag_out,
                         tri_sb, onc_sb, onr_sb,
                         qT_sb, kT_sb, v_sb, wopool, avpool, ppool,
                         srpool, sbpool, tspool, agpool, opool,
                         pmm2, psav, psrs)


def _phase1(nc, tc, xP, wqP, wkP, wvP, cosP, sinmP, eye_sb,
            qT_sb, kT_sb, v_sb, rcpool, wpool, xpool, rpool, pmm1, ptrp):
    # one dma_start per tensor: its descriptors fan out across all 16 DMA
    # rings, and sync-engine issue cost stays minimal (the scalar engine's
    # queue stalls DMAs behind compute-dependent copies).
    cos_sb = rcpool.tile([HD, S], BF16, tag="cos")
    sinm_sb = rcpool.tile([HD, S], BF16, tag="sinm")
    nc.sync.dma_start(out=cos_sb[:], in_=cosP[:])
    nc.sync.dma_start(out=sinm_sb[:], in_=sinmP[:])

    wq_sb = wpool.tile([128, NHT * 512], BF16, tag="wq")
    wk_sb = wpool.tile([128, NHT * 128], BF16, tag="wk")
    wv_sb = wpool.tile([128, NHT * 128], BF16, tag="wv")
    nc.sync.dma_start(out=wq_sb[:], in_=wqP[:])
    nc.sync.dma_start(out=wk_sb[:], in_=wkP[:])
    nc.sync.dma_start(out=wv_sb[:], in_=wvP[:])

    def _rope(dst, pp, sc):
        # dst[:, sc] = rot(pp): [0:64] = x1*c - x2*s ; [64:128] = x2*c + x1*s
        # cos_sb = [c; c], sinm_sb = [-s; s] (signs baked host-side)
        pq_sb = rpool.tile([128, 512], BF16, tag="pq")
        nc.scalar.copy(pq_sb[:], pp[:])
        sw = rpool.tile([128, 512], BF16, tag="sw")
        nc.vector.tensor_scalar_mul(sw[0:HALF, :], pq_sb[HALF:128, :], 1.0)
        nc.vector.tensor_scalar_mul(sw[HALF:128, :], pq_sb[0:HALF, :], 1.0)
        a = rpool.tile([128, 512], BF16, tag="ra")
        b = rpool.tile([128, 512], BF16, tag="rb")
        nc.vector.tensor_mul(a[:], pq_sb[:], cos_sb[:, sc])
        nc.vector.tensor_mul(b[:], sw[:], sinm_sb[:, sc])
        nc.vector.tensor_add(dst[:, sc], a[:], b[:])

    HW = NHT * 256  # 8192 cols per half-chunk
    for cs in range(NSC):
        sc = slice(cs * 512, (cs + 1) * 512)
        xh = [xpool.tile([128, HW], BF16, tag="x", name=f"x{cs}_{hf}")
              for hf in range(2)]
        for hf in range(2):
            nc.sync.dma_start(out=xh[hf][:],
                              in_=xP[:, (2 * cs + hf) * HW:
                                     (2 * cs + hf + 1) * HW])

        def xs(h):
            return xh[h // 16][:, (h % 16) * 512:(h % 16 + 1) * 512]

        # 6 accumulation chains interleaved across PSUM banks so one
        # matmul's drain overlaps the next one's fill
        pq = [pmm1.tile([128, 512], F32, tag="mm", name=f"pq{cs}_{j}")
              for j in range(NREP)]
        pk = pmm1.tile([128, 512], F32, tag="mm", name=f"pk{cs}")
        pv = pmm1.tile([128, 512], F32, tag="mm", name=f"pv{cs}")
        for h in range(NHT):
            st = (h == 0)
            sp = (h == NHT - 1)
            for j in range(NREP):
                nc.tensor.matmul(
                    pq[j][:],
                    wq_sb[:, h * 512 + j * 128: h * 512 + (j + 1) * 128],
                    xs(h), start=st, stop=sp)
            nc.tensor.matmul(pk[:], wk_sb[:, h * 128:(h + 1) * 128],
                             xs(h), start=st, stop=sp)
            nc.tensor.matmul(pv[:], wv_sb[:, h * 128:(h + 1) * 128],
                             xs(h), start=st, stop=sp)
        for j in range(NREP):
            _rope(qT_sb[j], pq[j], sc)
        _rope(kT_sb, pk, sc)
        # vT chunk [d, s'] then PE-transpose to [s, d]
        vt_sb = rpool.tile([128, 512], BF16, tag="vt", name=f"vt{cs}")
        nc.scalar.copy(vt_sb[:], pv[:])
        for tl in range(4):
            pt_ps = ptrp.tile([128, 128], BF16, tag="tr", name=f"vtr{cs}_{tl}")
            nc.tensor.transpose(pt_ps[:], vt_sb[:, tl * 128:(tl + 1) * 128],
                                eye_sb[:])
            t0 = (cs * 4 + tl) * 128
            nc.scalar.copy(v_sb[:, t0:t0 + 128], pt_ps[:])


def _phase23(nc, tc, woP, o_out, ag_in, ag_out,
             tri_sb, onc_sb, onr_sb, qT_sb, kT_sb, v_sb, wopool, avpool,
             ppool, srpool, sbpool, tspool, agpool, opool, pmm2, psav, psrs):
    Exp = mybir.ActivationFunctionType.Exp
    DEPTH = 3
    RG = [list(range(N_CORES))]

    wo_sb = wopool.tile([128, NHT * 512], BF16, tag="wo")
    nc.sync.dma_start(out=wo_sb[:], in_=woP[:])

    av_tiles = [avpool.tile([128, 2048], BF16, tag="av", name=f"av{C}")
                for C in range(NSC)]

    state = {"recip": None, "norm": None}
    rrecs = {}

    def make_recip(j, C, prs):
        def f():
            rrec = srpool.tile([1, 512], F32, tag="rrec", name=f"rrec{j}_{C}")
            nc.vector.reciprocal_approx_fast(rrec[:], prs[:])
            rb = srpool.tile([1, 512], BF16, tag="rrecb", name=f"rrb{j}_{C}")
            nc.vector.tensor_scalar_mul(rb[:], rrec[:], 1.0)
            rrecs[(j, C)] = rb
        return f

    def make_norm(j, C, pav):
        def f():
            rrec = rrecs.pop((j, C))
            pb = pmm2.tile([128, 512], F32, tag="mm", name=f"pb{j}_{C}")
            nc.tensor.matmul(pb[:], onr_sb[:], rrec[:], start=True, stop=True)
            bsb = sbpool.tile([128, 512], F32, tag="bsb", name=f"bsb{j}_{C}")
            nc.vector.tensor_scalar_mul(bsb[:], pb[:], 1.0)
            nc.vector.tensor_mul(av_tiles[C][:, j * 512:(j + 1) * 512],
                                 pav[:], bsb[:])
            if j == 3:
                # one AllGather per chunk: collectives serialize on the
                # gpsimd queue, so fewer+bigger is faster
                nc.sync.dma_start(out=ag_in[C][:], in_=av_tiles[C][:])
                nc.gpsimd.collective_compute(
                    "AllGather", mybir.AluOpType.bypass, replica_groups=RG,
                    ins=[ag_in[C][:]], outs=[ag_out[C][:]])
        return f

    def run_carry(kind):
        if state[kind] is not None:
            state[kind]()
            state[kind] = None

    def attn_unit(j, C):
        qc0 = C * 512
        nkt = 4 * C + 4
        pav = psav.tile([128, 512], F32, tag="av", name=f"pav{j}_{C}")
        prs = psrs.tile([1, 512], F32, tag="rs", name=f"prs{j}_{C}")
        tsum = tspool.tile([128, 512], F32, tag="ts", name=f"ts{j}_{C}")
        pend = []

        def drain_one():
            kt2, pt2 = pend.pop(0)
            nc.tensor.matmul(pav[:], v_sb[:, kt2 * 128:(kt2 + 1) * 128],
                             pt2[:], start=(kt2 == 0), stop=(kt2 == nkt - 1))

        for kt in range(nkt):
            off = max(0, (kt - 4 * C) * 128)
            ps = pmm2.tile([128, 512], F32, tag="mm", name=f"ps{j}_{C}_{kt}")
            nc.tensor.matmul(ps[:, off:512],
                             kT_sb[:, kt * 128:(kt + 1) * 128],
                             qT_sb[j][:, qc0 + off: qc0 + 512],
                             start=True, stop=True)
            pt = ppool.tile([128, 512], BF16, tag="pt", name=f"pt{j}_{C}_{kt}")
            if off > 0:
                nc.vector.memset(pt[:, 0:off], 0.0)
            nc.scalar.activation(pt[:, off:512], ps[:, off:512], Exp)
            if kt >= 4 * C:
                nc.vector.tensor_mul(pt[:, off:off + 128],
                                     pt[:, off:off + 128], tri_sb[:])
            # running tile-sum on DVE replaces the per-tile M=1 rowsum
            # matmul (one PE pass over every tile saved)
            if kt == 0:
                nc.vector.tensor_scalar_mul(tsum[:], pt[:], 1.0)
            else:
                nc.vector.tensor_add(tsum[:], tsum[:], pt[:])
            pend.append((kt, pt))
            if kt == 1:
                run_carry("recip")
            if kt == 3:
                run_carry("norm")
            if len(pend) > DEPTH:
                drain_one()
        while pend:
            drain_one()
        nc.tensor.matmul(prs[:], onc_sb[:], tsum[:], start=True, stop=True)
        state["recip"] = make_recip(j, C, prs)
        state["norm"] = make_norm(j, C, pav)

    def ph3(C):
        # readback 8 slabs [128, 2048]; slab r holds (j, r) tiles at
        # cols j*512 + q'; one sync dma_start each (16KB lines over rings)
        ts = []
        for r in range(N_CORES):
            t = agpool.tile([128, 2048], BF16, tag="ag", name=f"ag{C}_{r}")
            nc.sync.dma_start(out=t[:],
                              in_=ag_out[C][r * 128:(r + 1) * 128, :])
            ts.append(t)
        for stl in range(4):
            st = 4 * C + stl
            po = pmm2.tile([128, 512], F32, tag="mm", name=f"po{st}")
            n = 0
            for jgrp in range(NREP):
                for r in range(N_CORES):
                    nc.tensor.matmul(
                        po[:],
                        ts[r][:, jgrp * 512 + stl * 128:
                              jgrp * 512 + (stl + 1) * 128],
                        wo_sb[:, (jgrp * 8 + r) * 512:(jgrp * 8 + r + 1) * 512],
                        start=(n == 0), stop=(n == NREP * N_CORES - 1))
                    n += 1
            osb = opool.tile([128, 512], F32, tag="o", name=f"o{st}")
            nc.vector.tensor_scalar_mul(osb[:], po[:], 1.0)
            nc.sync.dma_start(out=o_out[st * 128:(st + 1) * 128, :],
                              in_=osb[:])

    # AG(C) fires in unit (C+1, 0)'s norm carry; the 4 AGs serialize on
    # gpsimd (~28us each), completing at roughly 44/72/100/128us into
    # phase 2 — consume each only after it has landed.
    PH3_AT = {(2, 2): 0, (3, 1): 1}
    for C in range(NSC):
        for j in range(NREP):
            attn_unit(j, C)
            if (C, j) in PH3_AT:
                ph3(PH3_AT[(C, j)])
    run_carry("recip")
    run_carry("norm")
    ph3(NSC - 2)
    ph3(NSC - 1)


def prep_inputs(hidden_states, wq, wk, wv, wo, cos, sin, causal_mask=None):
    bf16 = ml_dtypes.bfloat16
    x = np.asarray(hidden_states, np.float32)[0]          # (S, HID)
    xT = np.ascontiguousarray(x.T)                        # (HID, S)
    # chunk-major pack: xP[p, cs*16384 + h*512 + s'] = xT[h*128+p, cs*512+s']
    xP = np.ascontiguousarray(
        xT.reshape(NHT, 128, NSC, 512).transpose(1, 2, 0, 3)
        .reshape(128, NSC * NHT * 512)).astype(bf16)
    wq_s = np.asarray(wq, np.float32) / np.sqrt(HD)
    cos2 = np.asarray(cos, np.float32)[0, 0]              # (S, 64)
    sin2 = np.asarray(sin, np.float32)[0, 0]
    cosP = np.ascontiguousarray(
        np.concatenate([cos2.T, cos2.T], 0)).astype(bf16)  # [c; c]
    sinmP = np.ascontiguousarray(
        np.concatenate([-sin2.T, sin2.T], 0)).astype(bf16)  # [-s; s]
    kl = np.arange(128)[:, None]
    ql = np.arange(128)[None, :]
    triT = (kl <= ql).astype(bf16)
    eyeT = np.eye(128, dtype=np.float32).astype(bf16)
    ones_c = np.ones((128, 1), bf16)
    ones_r = np.ones((1, 128), np.float32)

    # wo reordered so row p = (j*8+r)*128 + d maps head (j*8+r), dim d
    j_ = np.arange(NREP)[:, None, None]
    r_ = np.arange(N_CORES)[None, :, None]
    d_ = np.arange(HD)[None, None, :]
    col_order = ((j_ * N_CORES + r_) * HD + d_).reshape(-1)
    woT_full = np.ascontiguousarray(
        np.asarray(wo, np.float32)[:, col_order].T)       # (4096 avrow, 4096)

    def pack_w(wT, ncols):  # (HID, ncols) -> (128, NHT*ncols), col h*ncols+c
        return np.ascontiguousarray(
            wT.reshape(NHT, 128, ncols).transpose(1, 0, 2)
            .reshape(128, NHT * ncols)).astype(bf16)

    in_maps = []
    for c in range(N_CORES):
        heads = [jj * N_CORES + c for jj in range(NREP)]
        wq_rows = np.concatenate([wq_s[h * HD:(h + 1) * HD, :] for h in heads],
                                 0)                        # (512, HID)
        wqP = pack_w(np.ascontiguousarray(wq_rows.T), 512)
        wkT_c = np.ascontiguousarray(
            np.asarray(wk, np.float32)[c * HD:(c + 1) * HD, :].T)
        wvT_c = np.ascontiguousarray(
            np.asarray(wv, np.float32)[c * HD:(c + 1) * HD, :].T)
        wkP = pack_w(wkT_c, 128)
        wvP = pack_w(wvT_c, 128)
        woP = pack_w(np.ascontiguousarray(
            woT_full[:, c * 512:(c + 1) * 512]), 512)
        in_maps.append(dict(xP=xP, wqP=wqP, wkP=wkP, wvP=wvP, woP=woP,
                            cosP=cosP, sinmP=sinmP, triT=triT, eyeT=eyeT,
                            ones_c=ones_c, ones_r=ones_r))
    return in_maps


def postprocess(results):
    out = np.empty((S, HID), np.float32)
    for c in range(N_CORES):
        out[:, c * 512:(c + 1) * 512] = results[c]["o_out"]
    return out[None]


def get_nc():
    if "nc" not in _CACHE:
        _CACHE["nc"] = build_nc()
    return _CACHE["nc"]


def kernel(hidden_states, wq, wk, wv, wo, cos, sin, causal_mask=None):
    nc = get_nc()
    in_maps = prep_inputs(hidden_states, wq, wk, wv, wo, cos, sin, causal_mask)
    res = run_bass_kernel_spmd(nc, in_maps, core_ids=list(range(N_CORES)))
    return postprocess(res.results)
